# revision 17
# baseline (speedup 1.0000x reference)
"""Trainium2 Bass kernel for nn_AttentionModule (dense transformer block).

Computation (per batch element b):
    q = X @ Wq.T ; k = K @ Wk.T ; v = X @ Wv.T        (X=query_input, K=key_input)
    a = softmax((k @ q.T) / sqrt(D), axis=-1)          -> (NK, NQ)
    out = a @ v + K                                    -> (NK, D)

Sharding: data-parallel over batch, one batch element per NeuronCore (B == 8).

Strategy:
  * The q/k projections are algebraically folded on the host:
        scores = K @ (Wk^T Wq) @ X^T = (K @ M) @ X^T
    so the device computes G = K@M (one 2048x1024x1024 GEMM) instead of the
    two projections q and k — 4.3 GFLOP saved per core.
  * All GEMMs run in fp8e4 with DoubleRow perf mode (two fp8 weights per PE
    cell, contraction 256 per instruction).  Operands live in single SBUF
    tiles of shape [128, n_kblocks, free] whose middle dim is the contraction
    block index; DoubleRow consumes [:, 2j:2j+2, :] slices.  M is pre-scaled
    by 16 and Wv by 32 on the host so their fp8 encodings stay in the normal
    range; the 1/16 folds into the exp() scale and the 1/32 into the softmax
    denominator ones vector (value 32), so no extra device work.
  * Scores are built in [n, m] layout so exp(S)^T feeds the context matmul
    directly as the stationary operand; the softmax denominator is a
    ones-vector DoubleRow matmul; the normalization and the +key_input
    residual are fused into one scalar_tensor_tensor on the vector engine.
  * Everything stays SBUF-resident.  DMAs are few and large (the HWDGE issue
    path costs ~650ns each regardless of size): 0.25-0.5MB column slices
    that land just ahead of the matmul groups that need them.  The residual
    and the output travel as fp16 (the fp32 result is rebuilt on the host);
    output stores issue from the sync/gpsimd queues because the ACT queue is
    saturated by the softmax exps.
"""

import numpy as np
import ml_dtypes

import concourse.tile as tile
from concourse import bacc, mybir
from concourse.bass_utils import run_bass_kernel_spmd
from concourse.masks import make_identity

B, NQ, NK, D = 8, 2048, 2048, 1024
P = 128
KB = D // P          # 8 contraction blocks of 128 (4 DoubleRow pairs)
JD = KB // 2         # 4 pair-blocks over d
NB = NQ // P         # 16 query-row blocks
JN = NB // 2         # 8 pair-blocks over n
MC = 512             # scores chunk width (n_k columns per chunk)
NMC = NK // MC       # 4 chunks
MSB = MC // P        # 4 output row blocks per chunk

M_SCALE = 16.0       # host pre-scale of M = Wk^T Wq   (fp8 range)
V_SCALE = 32.0       # host pre-scale of Wv^T          (fp8 range)
SCALE = 1.0 / float(np.sqrt(np.float32(D)))
EXP_SCALE = SCALE / M_SCALE          # undoes M_SCALE inside exp()
ONES_VAL = V_SCALE                   # folds 1/V_SCALE into 1/colsum

F32 = mybir.dt.float32
F16 = mybir.dt.float16
F8 = mybir.dt.float8e4
DR = mybir.MatmulPerfMode.DoubleRow

_CACHE = {}


def _build():
    nc = bacc.Bacc("TRN2", target_bir_lowering=False, debug=False, num_devices=B)

    # block layouts: t[p, k, c] = src[k*128 + p, c]
    xtp = nc.dram_tensor("xtp", [P, KB, NQ], F8, kind="ExternalInput").ap()
    ktp = nc.dram_tensor("ktp", [P, KB, NK], F8, kind="ExternalInput").ap()
    mp = nc.dram_tensor("mp", [P, KB, D], F8, kind="ExternalInput").ap()
    wvp = nc.dram_tensor("wvp", [P, KB, D], F8, kind="ExternalInput").ap()
    knat = nc.dram_tensor("knat", [NK, D], F16, kind="ExternalInput").ap()
    out = nc.dram_tensor("out", [NK, D], F16, kind="ExternalOutput").ap()

    with tile.TileContext(nc) as tc:
        with (
            tc.tile_pool(name="const", bufs=1) as constp,
            tc.tile_pool(name="inp", bufs=4) as inp,
            tc.tile_pool(name="gtp", bufs=1) as gtp,
            tc.tile_pool(name="vp", bufs=1) as vp,
            tc.tile_pool(name="epp", bufs=2) as epp,
            tc.tile_pool(name="small", bufs=4) as smallp,
            tc.tile_pool(name="knp", bufs=8) as knp,
            tc.tile_pool(name="outp", bufs=8) as outp,
            tc.tile_pool(name="psum", bufs=1, space="PSUM") as psp,
        ):
            ident = constp.tile([1, 1], F32, tag="ident", name="ident")
            make_identity(nc, ident)
            # pair stride of the [:, :, 0:1] slice must be 16B-aligned
            ones = constp.tile([P, 2, 16], F8, tag="ones", name="ones")
            nc.vector.memset(ones, ONES_VAL)

            mt = inp.tile([P, KB, D], F8, tag="m", name="mt", bufs=1)
            ktt = inp.tile([P, KB, NK], F8, tag="kt", name="ktt", bufs=1)
            wvt = inp.tile([P, KB, D], F8, tag="wv", name="wvt", bufs=1)
            xtt = inp.tile([P, KB, NQ], F8, tag="xt", name="xtt", bufs=1)
            # one 0.25-0.5MB DMA per column slice covers all 8 k-blocks
            nc.sync.dma_start(out=mt[:, :, 0:512], in_=mp[:, :, 0:512])
            nc.sync.dma_start(out=ktt[:, :, 0:512], in_=ktp[:, :, 0:512])
            nc.sync.dma_start(out=mt[:, :, 512:D], in_=mp[:, :, 512:D])
            for q in range(1, NMC):
                nc.sync.dma_start(
                    out=ktt[:, :, q * 512:(q + 1) * 512],
                    in_=ktp[:, :, q * 512:(q + 1) * 512],
                )
            nc.sync.dma_start(out=wvt, in_=wvp)
            for q in range(NQ // 512):
                nc.sync.dma_start(
                    out=xtt[:, :, q * 512:(q + 1) * 512],
                    in_=xtp[:, :, q * 512:(q + 1) * 512],
                )

            gt = gtp.tile([P, KB, NK], F8, tag="gt", name="gt")
            vt = vp.tile([P, NB, D], F8, tag="v", name="vt")

            ci = 0

            def psum_copy(dst, src):
                # alternate PSUM->SBUF copies between the two engines that
                # have a PSUM port so neither becomes the phase bottleneck
                nonlocal ci
                ci += 1
                if ci % 2 == 0:
                    nc.vector.tensor_copy(dst, src)
                else:
                    nc.scalar.activation(
                        out=dst, in_=src, func=mybir.ActivationFunctionType.Copy
                    )

            # ---------------- phase A: G^T = M^T-pairs @ K^T ----------------
            # gt[:, e, m] = sum_d M[d, e*128..] * K^T[d, m]
            for q in range(NMC):
                for e in range(KB):
                    ps = psp.tile([P, 512], F32, tag="mm", name="mm", bufs=4)
                    for jd in range(JD):
                        nc.tensor.matmul(
                            ps,
                            mt[:, 2 * jd:2 * jd + 2, e * P:(e + 1) * P],
                            ktt[:, 2 * jd:2 * jd + 2, q * 512:(q + 1) * 512],
                            start=(jd == 0),
                            stop=(jd == JD - 1),
                            perf_mode=DR,
                        )
                    psum_copy(gt[:, e, q * 512:(q + 1) * 512], ps)

            # ---------------- phase B: V~ = X @ (32 Wv)^T ----------------
            # vt[:, nb, dv] = sum_d X[nb-rows, d] * 32*Wv[dv, d]
            for nb in range(NB):
                for dc in range(D // 512):
                    ps = psp.tile([P, 512], F32, tag="mm", name="mm", bufs=4)
                    for jd in range(JD):
                        nc.tensor.matmul(
                            ps,
                            xtt[:, 2 * jd:2 * jd + 2, nb * P:(nb + 1) * P],
                            wvt[:, 2 * jd:2 * jd + 2, dc * 512:(dc + 1) * 512],
                            start=(jd == 0),
                            stop=(jd == JD - 1),
                            perf_mode=DR,
                        )
                    psum_copy(vt[:, nb, dc * 512:(dc + 1) * 512], ps)

            # ---------------- phase C: chunked attention ----------------
            # For chunk c: S^T[n, m] -> exp -> colsum -> context + residual.
            # The softmax tail (last colsum, reciprocal, transpose) of chunk c
            # is emitted two S-groups into chunk c+1 and the context of chunk
            # c after all of S(c+1), so the PE never waits on ACT/DVE.
            state = {}

            def emit_tail(c):
                cs_ps, ept = state[c]["cs"], state[c]["ep"]
                nc.tensor.matmul(
                    cs_ps, ones[:, :, 0:1], ept[:, NB - 2:NB, :],
                    start=False, stop=True, perf_mode=DR,
                )
                recip = smallp.tile([1, MC], F32, tag="rr", name="rr", bufs=2)
                nc.vector.reciprocal(recip, cs_ps)
                rp_ps = psp.tile([P, MSB], F32, tag="csrp", name="rp", bufs=1)
                for j in range(MSB):
                    nc.tensor.transpose(
                        rp_ps[:, j:j + 1], recip[:, j * P:(j + 1) * P], ident
                    )
                rpp = smallp.tile([P, MSB], F32, tag="rpp", name="rpp", bufs=2)
                nc.vector.tensor_copy(rpp, rp_ps)
                state[c]["rpp"] = rpp

            def emit_ctx(c, after_first_group=None):
                # after_first_group: emitted between the first context matmul
                # group and its STT — used for the last chunk to slot the
                # softmax tail (colsum #7 / reciprocal / transpose) behind
                # useful PE work instead of stalling on the final exp.
                ept, kns = state[c]["ep"], state[c]["kn"]
                m0 = c * MC
                for msb in range(MSB):
                    for dc in range(D // 512):
                        ps = psp.tile([P, 512], F32, tag="ctx", name="ctx", bufs=3)
                        for jn in range(JN):
                            nc.tensor.matmul(
                                ps,
                                ept[:, 2 * jn:2 * jn + 2, msb * P:(msb + 1) * P],
                                vt[:, 2 * jn:2 * jn + 2, dc * 512:(dc + 1) * 512],
                                start=(jn == 0),
                                stop=(jn == JN - 1),
                                perf_mode=DR,
                            )
                        if after_first_group is not None:
                            after_first_group()
                            after_first_group = None
                        ot = outp.tile([P, 512], F16, tag="ot", name="ot", bufs=8)
                        nc.vector.scalar_tensor_tensor(
                            out=ot,
                            in0=ps,
                            scalar=state[c]["rpp"][:, msb:msb + 1],
                            in1=kns[msb][:, dc * 512:(dc + 1) * 512],
                            op0=mybir.AluOpType.mult,
                            op1=mybir.AluOpType.add,
                        )
                        # stores avoid the ACT queue (it is saturated by the
                        # exps) and alternate sync/gpsimd so neither issue
                        # path becomes the store-cadence limiter
                        eng = nc.sync if (msb * 2 + dc) % 2 == 1 else nc.gpsimd
                        eng.dma_start(
                            out=out[m0 + msb * P:m0 + (msb + 1) * P,
                                    dc * 512:(dc + 1) * 512],
                            in_=ot,
                        )
                del state[c]

            for c in range(NMC):
                m0 = c * MC
                kns = []
                for msb in range(MSB):
                    t = knp.tile([P, D], F16, tag="kn", name="kn", bufs=8)
                    nc.sync.dma_start(
                        out=t, in_=knat[m0 + msb * P:m0 + (msb + 1) * P, :]
                    )
                    kns.append(t)
                ept = epp.tile([P, NB, MC], F8, tag="ep", name="ep", bufs=2)
                cs_ps = psp.tile([1, MC], F32, tag="csrp", name="cs", bufs=1)
                state[c] = {"cs": cs_ps, "ep": ept, "kn": kns}

                for nb in range(NB):
                    ps = psp.tile([P, MC], F32, tag="mm", name="mm", bufs=4)
                    for jd in range(JD):
                        nc.tensor.matmul(
                            ps,
                            xtt[:, 2 * jd:2 * jd + 2, nb * P:(nb + 1) * P],
                            gt[:, 2 * jd:2 * jd + 2, m0:m0 + MC],
                            start=(jd == 0),
                            stop=(jd == JD - 1),
                            perf_mode=DR,
                        )
                    nc.scalar.activation(
                        out=ept[:, nb, :], in_=ps,
                        func=mybir.ActivationFunctionType.Exp, scale=EXP_SCALE,
                    )
                    if nb == 1 and c > 0:
                        emit_tail(c - 1)
                    # colsum for pair jn lags its exps by two S-groups so the
                    # exp -> colsum semaphore never gates the PE
                    if nb >= 3 and nb % 2 == 1:
                        jn = (nb - 3) // 2
                        nc.tensor.matmul(
                            cs_ps, ones[:, :, 0:1], ept[:, 2 * jn:2 * jn + 2, :],
                            start=(jn == 0), stop=False, perf_mode=DR,
                        )
                if c > 0:
                    emit_ctx(c - 1)
            emit_ctx(NMC - 1, after_first_group=lambda: emit_tail(NMC - 1))

    nc.compile()
    return nc


def _get_nc():
    if "nc" not in _CACHE:
        _CACHE["nc"] = _build()
    return _CACHE["nc"]


def _blk(a):
    """[D, C] -> [P, D//P, C] with t[p, k, c] = a[k*128 + p, c]."""
    Dd, C = a.shape
    return np.ascontiguousarray(a.reshape(Dd // P, P, C).transpose(1, 0, 2))


def make_in_maps(query_input, key_input, Wq, Wk, Wv):
    f8 = ml_dtypes.float8_e4m3
    query_input = np.asarray(query_input, dtype=np.float32)
    key_input = np.asarray(key_input, dtype=np.float32)
    Wq = np.asarray(Wq, dtype=np.float32)
    Wk = np.asarray(Wk, dtype=np.float32)
    Wv = np.asarray(Wv, dtype=np.float32)

    m_pre = _blk(M_SCALE * (Wk.T @ Wq)).astype(f8)
    wv_pre = _blk(V_SCALE * Wv.T).astype(f8)
    in_maps = []
    for b in range(B):
        in_maps.append({
            "xtp": _blk(query_input[b].T.copy()).astype(f8),
            "ktp": _blk(key_input[b].T.copy()).astype(f8),
            "mp": m_pre,
            "wvp": wv_pre,
            "knat": np.ascontiguousarray(key_input[b]).astype(np.float16),
        })
    return in_maps


def kernel(query_input, key_input, Wq, Wk, Wv):
    nc = _get_nc()
    in_maps = make_in_maps(query_input, key_input, Wq, Wk, Wv)
    res = run_bass_kernel_spmd(nc, in_maps, list(range(B))).results
    return np.stack([res[b]["out"] for b in range(B)], axis=0).astype(np.float32)
